# revision 1
# baseline (speedup 1.0000x reference)
"""Local (banded, window=3) attention TRN2 kernel.

Full-input contract: kernel(**inputs) takes the complete tensors
  x [8, 1024, 384], qkv_w [1152, 384], proj_w [384, 384], proj_b [384]
and returns the full output [8, 1024, 384].

Sharding: data-parallel over batch B=8 -> one batch element per NeuronCore.

Per-core algorithm (bf16 data, fp32 PSUM accumulation, fp32 softmax):
  xT [C=384, N=1024] (host-pretransposed shard, bf16)
  qkvT[ch, t] = qkv_w @ x_b.T          (PE; lhsT = host-pretransposed qkv_w.T)
  band scores s_off[h, t] = sum_d q[(h,d),t] * k[(h,d),t+off], off in {-1,0,+1}
     products on DVE in [ch, t] layout (token shift = free-dim slice),
     partition-reduction over d via PE matmul against a 0/1 head-indicator
  p = softmax over the 3 offsets (ACT exp, DVE add / recip-approx / mul)
  attn_outT[(h,d), t] = sum_off pbcast_off[(h,d), t] * vT[(h,d), t+off]
     (p broadcast head->64 rows via PE indicator matmul into PSUM,
      multiply-add on DVE reading PSUM directly)
  yT = proj_w @ attn_outT + b     (PE; bias folded in as a K=1 matmul)
Host transposes yT back to [1024, 384] fp32 per batch element.
"""

import numpy as np

B, N, C = 8, 1024, 384
H, HD = 6, 64
CQKV = 3 * C  # 1152
NCORES = 8
P = 128
NHALF = N // 2  # 512
KC = C // P  # 3 contraction chunks

_cached = {}


def _build_nc():
    import contextlib

    import concourse.bacc as bacc
    import concourse.tile as tile
    from concourse import mybir

    f32 = mybir.dt.float32
    bf16 = mybir.dt.bfloat16
    AF = mybir.ActivationFunctionType

    nc = bacc.Bacc("TRN2", target_bir_lowering=False, debug=False,
                   num_devices=NCORES)

    d_xT = nc.dram_tensor("xT", [C, N], bf16, kind="ExternalInput").ap()
    d_wqkvT = nc.dram_tensor("qkv_wT", [C, CQKV], bf16,
                             kind="ExternalInput").ap()
    d_wprojT = nc.dram_tensor("proj_wT", [C, C], bf16,
                              kind="ExternalInput").ap()
    d_bias = nc.dram_tensor("proj_b", [1, C], bf16, kind="ExternalInput").ap()
    d_ind6 = nc.dram_tensor("ind6", [P, 6 * KC], bf16,
                            kind="ExternalInput").ap()
    d_ind6T = nc.dram_tensor("ind6T", [H, P * KC], bf16,
                             kind="ExternalInput").ap()
    d_ones = nc.dram_tensor("ones", [1, N], bf16, kind="ExternalInput").ap()
    d_yT = nc.dram_tensor("yT", [C, N], bf16, kind="ExternalOutput").ap()

    with tile.TileContext(nc) as tc, contextlib.ExitStack() as ctx:
        wpool = ctx.enter_context(tc.tile_pool(name="w", bufs=1))
        xpool = ctx.enter_context(tc.tile_pool(name="x", bufs=1))
        qkvpool = ctx.enter_context(tc.tile_pool(name="qkv", bufs=1))
        prodpool = ctx.enter_context(tc.tile_pool(name="prod", bufs=12))
        avpool = ctx.enter_context(tc.tile_pool(name="av", bufs=10))
        aopool = ctx.enter_context(tc.tile_pool(name="ao", bufs=1))
        ypool = ctx.enter_context(tc.tile_pool(name="y", bufs=4))
        epool = ctx.enter_context(tc.tile_pool(name="e", bufs=20))
        # PSUM budget (8 banks of 512 fp32):
        #   mm   [128, 512] = 1 bank x 4 bufs = 4  (stage-1 qkv)
        #   pb   [128, 512]  = 1 bank  x 2 bufs = 2  (p-broadcast + proj)
        #   s    [6, 512]    = 1 bank  x 2 bufs = 2  (scores)
        mmpool = ctx.enter_context(
            tc.tile_pool(name="mm", bufs=4, space="PSUM"))
        pbpool = ctx.enter_context(
            tc.tile_pool(name="pb", bufs=2, space="PSUM"))
        spool = ctx.enter_context(
            tc.tile_pool(name="s", bufs=2, space="PSUM"))

        # ---- inputs: per-chunk DMAs on both HWDGE queues (sync + scalar)
        # so stage-1 matmuls of chunk kc can start as soon as x[kc]/w[kc] land
        w_qkv, w_proj, x_t = [], [], []
        for kc in range(KC):
            xt = xpool.tile([P, N], bf16, name=f"xT{kc}")
            if kc == 0:
                # split so the first stage-1 matmul group starts sooner
                nc.sync.dma_start(out=xt[:, 0:NHALF],
                                  in_=d_xT[0:P, 0:NHALF])
                nc.sync.dma_start(out=xt[:, NHALF:N],
                                  in_=d_xT[0:P, NHALF:N])
            else:
                nc.sync.dma_start(out=xt, in_=d_xT[P * kc:P * (kc + 1), :])
            x_t.append(xt)
            wt = wpool.tile([P, CQKV], bf16, name=f"wqkv{kc}")
            # split per q/k/v part so the first stage-1 matmuls (q chunks)
            # start as soon as the q-part of the weights lands
            for part in range(3):
                nc.scalar.dma_start(
                    out=wt[:, C * part:C * (part + 1)],
                    in_=d_wqkvT[P * kc:P * (kc + 1), C * part:C * (part + 1)])
            w_qkv.append(wt)
        ind6 = wpool.tile([P, 6 * KC], bf16, name="ind6")
        nc.sync.dma_start(out=ind6, in_=d_ind6)
        ind6T = wpool.tile([H, P * KC], bf16, name="ind6T")
        nc.sync.dma_start(out=ind6T, in_=d_ind6T)
        for kc in range(KC):
            pt = wpool.tile([P, C], bf16, name=f"wproj{kc}")
            nc.scalar.dma_start(out=pt, in_=d_wprojT[P * kc:P * (kc + 1), :])
            w_proj.append(pt)
        bias = wpool.tile([1, C], bf16, name="bias")
        nc.sync.dma_start(out=bias, in_=d_bias)
        ones = wpool.tile([1, N], bf16, name="ones")
        nc.sync.dma_start(out=ones, in_=d_ones)

        def stage1_chunk(m, evac_engine):
            """qkvT[m] [128,1024] bf16 = (qkv_w @ x.T) rows 128m..128m+127."""
            qt = qkvpool.tile([P, N], bf16, name=f"qkvT{m}")
            for h in range(2):
                ps = mmpool.tile([P, NHALF], f32, tag="mm")
                for kc in range(KC):
                    nc.tensor.matmul(
                        ps,
                        lhsT=w_qkv[kc][:, P * m:P * (m + 1)],
                        rhs=x_t[kc][:, NHALF * h:NHALF * (h + 1)],
                        start=(kc == 0), stop=(kc == KC - 1),
                    )
                dst = qt[:, NHALF * h:NHALF * (h + 1)]
                nc.scalar.copy(dst, ps)
            return qt

        # ---- stage 1: all 9 qkvT chunks (PE warms up on a dense stream) ----
        # q/k interleaved so prods of chunk kc can start after 2 chunks;
        # v chunks are emitted later (after the score matmuls) so the PE has
        # work queued while the softmax chain runs on ACT/DVE
        qkvT = [None] * 9
        for m in (0, 3, 1, 4, 2, 5):
            qkvT[m] = stage1_chunk(m, "act")

        def half(ap, h):
            return ap[:, NHALF * h:NHALF * (h + 1)]


        # ---- banded attention mid-section, pipelined as 2 half-lanes ----
        # offsets: 0 -> key j=t-1, 1 -> j=t, 2 -> j=t+1
        def make_prod(off, kc):
            """prod[off][kc] [128, 1024] = q * shifted k (DVE, bf16)."""
            q = qkvT[kc]
            k = qkvT[3 + kc]
            pr = prodpool.tile([P, N], bf16, tag="prod",
                               name=f"prod{off}_{kc}")
            if off == 0:
                # col 0 unwritten: masked after exp via e[0] col 0
                nc.vector.tensor_mul(pr[:, 1:], q[:, 1:], k[:, 0:N - 1])
            elif off == 1:
                nc.vector.tensor_mul(pr, q, k)
            else:
                # col N-1 unwritten: masked after exp via e[2] col N-1
                nc.vector.tensor_mul(pr[:, 0:N - 1], q[:, 0:N - 1], k[:, 1:N])
            return pr

        prods = [[make_prod(off, kc) for kc in range(KC)] for off in range(3)]
        e_half = [[None] * 3 for _ in range(2)]   # [h][off]
        for h in range(2):
            for off in range(3):
                sps = spool.tile([H, NHALF], f32, tag="s")
                for kc in range(KC):
                    nc.tensor.matmul(
                        sps,
                        lhsT=ind6[:, 6 * kc:6 * (kc + 1)],
                        rhs=prods[off][kc][:, NHALF * h:NHALF * (h + 1)],
                        start=(kc == 0), stop=(kc == KC - 1),
                    )
                et = epool.tile([H, NHALF], f32, tag="e", name=f"e{h}_{off}")
                with tc.high_priority():
                    nc.scalar.activation(et, sps, AF.Exp,
                                         scale=float(HD) ** -0.5)
                e_half[h][off] = et

        # boundary masking: no left neighbor at t=0, no right at t=N-1
        nc.gpsimd.memset(e_half[0][0][:, 0:1], 0.0)
        nc.gpsimd.memset(e_half[1][2][:, NHALF - 1:NHALF], 0.0)

        # ---- stage 1 v chunks: PE work overlapping the softmax chain ----
        for m in (6, 7, 8):
            qkvT[m] = stage1_chunk(m, "act")

        # dL[t] = v[t-1] - v[t]  (padded: dL[0] = dL[N] = 0). AV then becomes
        # attn = v + p_l*dL - p_r*shift(dL), using that p_l + p_c + p_r = 1.
        dLs = []
        for kc in range(KC):
            v = qkvT[6 + kc]
            dL = avpool.tile([P, N + 1], bf16, tag="dv", name=f"dL{kc}")
            nc.vector.memset(dL[:, 0:1], 0.0)
            nc.vector.memset(dL[:, N:N + 1], 0.0)
            nc.vector.tensor_sub(dL[:, 1:N], v[:, 0:N - 1], v[:, 1:N])
            dLs.append(dL)

        # ---- softmax over the 3 offsets (per half) ----
        p_half = [[None] * 3 for _ in range(2)]
        for h in range(2):
            e0, e1, e2 = e_half[h]
            with tc.high_priority():
                den0 = epool.tile([H, NHALF], f32, tag="e")
                nc.vector.tensor_add(den0, e0, e1)
                den = epool.tile([H, NHALF], f32, tag="e")
                nc.vector.tensor_add(den, den0, e2)
                rec = epool.tile([H, NHALF], f32, tag="e")
                nc.vector.reciprocal_approx_fast(out=rec, in_=den)
                for off in (0, 2):
                    pt = epool.tile([H, NHALF], bf16, tag="p",
                                    name=f"p{h}_{off}")
                    nc.vector.tensor_mul(pt, e_half[h][off], rec)
                    p_half[h][off] = pt

        # ---- p broadcast (PE) + AV (DVE) + interleaved projection ----
        # proj accumulates over kc as soon as attn[kc] of this half exists,
        # so the PE overlaps the AV chain. yps tiles reuse the (now idle)
        # stage-1 mm PSUM pool.
        attn = [aopool.tile([P, N], bf16, name=f"attn{kc}")
                for kc in range(KC)]
        for h in range(2):
            lo = NHALF * h
            hi = lo + NHALF
            yps = [mmpool.tile([P, NHALF], f32, tag="mm", name=f"y{m}_{h}")
                   for m in range(KC)]
            for kc in range(KC):
                v = qkvT[6 + kc]

                def bcast(off, _h=h, _kc=kc):
                    # one pb PSUM tile live at a time (pb pool: 2 slots).
                    # Lane 0: DVE multiplies straight from PSUM (1x read).
                    # Lane 1: ACT evacuates to bf16 SBUF first so the DVE
                    # multiply runs in 2x mode - balances ACT vs DVE load.
                    pbps = pbpool.tile([P, NHALF], f32, tag="pb",
                                       name=f"pb{_kc}_{off}_{_h}")
                    nc.tensor.matmul(
                        pbps,
                        lhsT=ind6T[:, P * _kc:P * (_kc + 1)],
                        rhs=p_half[_h][off],
                        start=True, stop=True,
                    )
                    if _h == 0:
                        return pbps
                    pbs = avpool.tile([P, NHALF], bf16, tag="pbs")
                    nc.scalar.copy(pbs, pbps)
                    return pbs

                dL = dLs[kc]
                pb = bcast(0)
                m1 = avpool.tile([P, NHALF], bf16, tag="m")
                nc.vector.tensor_mul(m1, pb, dL[:, lo:hi])
                pb = bcast(2)
                m2 = avpool.tile([P, NHALF], bf16, tag="m")
                nc.vector.tensor_mul(m2, pb, dL[:, lo + 1:hi + 1])
                s12 = avpool.tile([P, NHALF], bf16, tag="m")
                nc.vector.tensor_sub(s12, m1, m2)
                nc.vector.tensor_add(half(attn[kc], h), s12, v[:, lo:hi])

                for m in range(KC):
                    nc.tensor.matmul(
                        yps[m],
                        lhsT=w_proj[kc][:, P * m:P * (m + 1)],
                        rhs=half(attn[kc], h),
                        start=(kc == 0), stop=False,
                    )

            for m in range(KC):
                nc.tensor.matmul(
                    yps[m],
                    lhsT=bias[:, P * m:P * (m + 1)],
                    rhs=half(ones, h),
                    start=False, stop=True,
                )
                yt = ypool.tile([P, NHALF], bf16, tag="y")
                nc.scalar.copy(yt, yps[m])
                nc.sync.dma_start(
                    out=d_yT[P * m:P * (m + 1), NHALF * h:NHALF * (h + 1)],
                    in_=yt)

    nc.compile()
    return nc


def _host_inputs(x, qkv_w, proj_w, proj_b):
    import ml_dtypes
    bf = ml_dtypes.bfloat16

    qkv_wT = np.ascontiguousarray(qkv_w.astype(np.float32).T).astype(bf)
    proj_wT = np.ascontiguousarray(proj_w.astype(np.float32).T).astype(bf)
    bias = proj_b.astype(np.float32).reshape(1, C).astype(bf)
    # head indicator: row p of chunk kc belongs to head 2*kc + p//64
    ind6 = np.zeros((P, 6 * KC), np.float32)
    ind6T = np.zeros((H, P * KC), np.float32)
    for kc in range(KC):
        for p in range(P):
            ind6[p, 6 * kc + 2 * kc + p // HD] = 1.0
            ind6T[2 * kc + p // HD, P * kc + p] = 1.0
    shared = {
        "qkv_wT": qkv_wT,
        "proj_wT": proj_wT,
        "proj_b": bias,
        "ind6": ind6.astype(bf),
        "ind6T": ind6T.astype(bf),
        "ones": np.ones((1, N), bf),
    }
    in_maps = []
    for b in range(B):
        m = dict(shared)
        m["xT"] = np.ascontiguousarray(x[b].astype(np.float32).T).astype(bf)
        in_maps.append(m)
    return in_maps


def kernel(x, qkv_w, proj_w, proj_b, _trace=False):
    from concourse import bass_utils

    x = np.asarray(x)
    if "nc" not in _cached:
        _cached["nc"] = _build_nc()
    nc = _cached["nc"]
    in_maps = _host_inputs(x, np.asarray(qkv_w), np.asarray(proj_w),
                           np.asarray(proj_b))
    res = bass_utils.run_bass_kernel_spmd(
        nc, in_maps, core_ids=list(range(NCORES)), trace=_trace)
    out = np.empty((B, N, C), np.float32)
    for b in range(B):
        out[b] = res.results[b]["yT"].astype(np.float32).T
    if _trace:
        _cached["last_result"] = res
    return out



# revision 12
# speedup vs baseline: 1.0700x; 1.0700x over previous
"""Local (banded, window=3) attention TRN2 kernel — v2 (PE-dense schedule).

Full-input contract: kernel(**inputs) takes the complete tensors
  x [8, 1024, 384], qkv_w [1152, 384], proj_w [384, 384], proj_b [384]
and returns the full output [8, 1024, 384].

Sharding: data-parallel over batch B=8 -> one batch element per NeuronCore.

Performance structure (vs the v1 baseline):
  - host repacks x / qkv_w / proj_w into SBUF-tile layout so inputs land
    with a few large fully-contiguous DMAs (2KB+ bursts)
  - PE warm-up: dummy matmuls ramp the PE p-state toward full clock
    while the input DMAs land (TRN2 runs 0.65/1.2/2.4 GHz depending on
    how long the PE has been continuously busy; idle gaps reset it)
  - one dense PE stream: qk projection -> v chunks interleaved with
    band-score matmuls -> softmax denominator reduced on the PE (exp
    stripes packed at partitions 0/32/64, summed by one indicator
    matmul) -> p-broadcast + output projection interleaved with AV
  - proj bias folded into the PSUM->SBUF output evacuation on ACT
    (Identity activation with a per-partition bias AP); no bias matmuls
  - engine split: ACT = qk evacs + exps + masks + y out; DVE = prods,
    v evacs, recip, p-muls, AV chain (broadcast read straight from
    PSUM); Pool = dL chains (GPSIMD cannot touch PSUM)
"""

import os

import numpy as np

KDEBUG = bool(int(os.environ.get("KDEBUG", "0")))

B, N, C = 8, 1024, 384
H, HD = 6, 64
CQKV = 3 * C  # 1152
NCORES = 8
P = 128
NHALF = N // 2  # 512
KC = C // P  # 3 contraction chunks
STAGE_M = (0, 3, 1, 4, 2, 5, 6, 7, 8)  # stage-1 output-chunk order (q/k first)
N_WARM = 6  # dummy matmuls to ramp the PE while input DMAs land

_cached = {}


def _build_nc():
    import contextlib

    import concourse.bacc as bacc
    import concourse.tile as tile
    from concourse import mybir

    f32 = mybir.dt.float32
    bf16 = mybir.dt.bfloat16
    AF = mybir.ActivationFunctionType

    nc = bacc.Bacc("TRN2", target_bir_lowering=False, debug=False,
                   num_devices=NCORES)

    d_x = nc.dram_tensor("xp", [P, KC * N], bf16, kind="ExternalInput").ap()
    d_wq = nc.dram_tensor("wqp", [P, 9 * KC * P], bf16,
                          kind="ExternalInput").ap()
    d_wp = nc.dram_tensor("wpp", [P, KC * KC * P], bf16,
                          kind="ExternalInput").ap()
    d_ind6 = nc.dram_tensor("ind6", [P, 6 * KC], bf16,
                            kind="ExternalInput").ap()
    d_ind6T = nc.dram_tensor("ind6T", [H, P * KC], bf16,
                             kind="ExternalInput").ap()
    d_indA = nc.dram_tensor("indA", [38, H], f32, kind="ExternalInput").ap()
    d_i6 = nc.dram_tensor("i6", [H, H], f32, kind="ExternalInput").ap()
    d_biasT = nc.dram_tensor("biasT", [P, KC], f32,
                             kind="ExternalInput").ap()
    d_yT = nc.dram_tensor("yT", [C, N], bf16, kind="ExternalOutput").ap()
    if KDEBUG:
        d_dbg_e = [nc.dram_tensor(f"dbg_e{h}", [44, NHALF], f32,
                                  kind="ExternalOutput").ap()
                   for h in range(2)]
        d_dbg_rec = [nc.dram_tensor(f"dbg_rec{h}", [H, NHALF], f32,
                                    kind="ExternalOutput").ap()
                     for h in range(2)]
        d_dbg_p = [nc.dram_tensor(f"dbg_p{h}", [2 * H, NHALF], bf16,
                                  kind="ExternalOutput").ap()
                   for h in range(2)]
        d_dbg_attn = nc.dram_tensor("dbg_attn", [C, N], bf16,
                                    kind="ExternalOutput").ap()
        d_dbg_qkv = nc.dram_tensor("dbg_qkv", [CQKV, N], bf16,
                                   kind="ExternalOutput").ap()

    with tile.TileContext(nc) as tc, contextlib.ExitStack() as ctx:
        wpool = ctx.enter_context(tc.tile_pool(name="w", bufs=1))
        xpool = ctx.enter_context(tc.tile_pool(name="x", bufs=1))
        qkvpool = ctx.enter_context(tc.tile_pool(name="qkv", bufs=1))
        prodpool = ctx.enter_context(tc.tile_pool(name="prod", bufs=9))
        avpool = ctx.enter_context(tc.tile_pool(name="av", bufs=4))
        aopool = ctx.enter_context(tc.tile_pool(name="ao", bufs=1))
        ypool = ctx.enter_context(tc.tile_pool(name="y", bufs=1))
        epool = ctx.enter_context(tc.tile_pool(name="e", bufs=2))
        # PSUM budget (8 banks of 512 fp32):
        #   mm [128,512] x3 (stage-1 qkv, then the proj accumulators)
        #   pb [128,512] x3 (warmup, then p-broadcast — read by DVE)
        #   s  [70,512]  x2 (scores / softmax denominator)
        mmpool = ctx.enter_context(
            tc.tile_pool(name="mm", bufs=3, space="PSUM"))
        pbpool = ctx.enter_context(
            tc.tile_pool(name="pb", bufs=3, space="PSUM"))
        spool = ctx.enter_context(
            tc.tile_pool(name="s", bufs=2, space="PSUM"))

        # ---- input DMAs: big contiguous transfers on the two HWDGE queues
        xt = xpool.tile([P, KC * N], bf16, name="xt")
        # first halves of all three c-chunks first so the first stage-1
        # group (h=0) can start as soon as possible
        nc.sync.dma_start(
            out=xt.rearrange("p (k n) -> p k n", k=KC)[:, :, 0:NHALF],
            in_=d_x.rearrange("p (k n) -> p k n", k=KC)[:, :, 0:NHALF])
        nc.sync.dma_start(
            out=xt.rearrange("p (k n) -> p k n", k=KC)[:, :, NHALF:N],
            in_=d_x.rearrange("p (k n) -> p k n", k=KC)[:, :, NHALF:N])

        wqt = wpool.tile([P, 9 * KC * P], bf16, name="wqt")
        nc.scalar.dma_start(out=wqt[:, 0:KC * P], in_=d_wq[:, 0:KC * P])
        nc.scalar.dma_start(out=wqt[:, KC * P:2 * KC * P],
                            in_=d_wq[:, KC * P:2 * KC * P])
        nc.scalar.dma_start(out=wqt[:, 2 * KC * P:], in_=d_wq[:, 2 * KC * P:])

        ind6 = wpool.tile([P, 6 * KC], bf16, name="ind6")
        nc.sync.dma_start(out=ind6, in_=d_ind6)
        indA = wpool.tile([38, H], f32, name="indA")
        nc.sync.dma_start(out=indA, in_=d_indA)
        i6 = wpool.tile([H, H], f32, name="i6")
        nc.sync.dma_start(out=i6, in_=d_i6)
        ind6T = wpool.tile([H, P * KC], bf16, name="ind6T")
        nc.sync.dma_start(out=ind6T, in_=d_ind6T)
        biasT = wpool.tile([P, KC], f32, name="biasT")
        nc.sync.dma_start(out=biasT, in_=d_biasT)
        wpt = wpool.tile([P, KC * KC * P], bf16, name="wpt")
        nc.scalar.dma_start(out=wpt, in_=d_wp)

        # ---- PE warm-up: ramp the p-state while the input DMAs land ----
        warm = wpool.tile([P, NHALF], bf16, name="warm")
        nc.gpsimd.memset(warm, 0.0)
        for _ in range(N_WARM):
            wps = pbpool.tile([P, NHALF], f32, tag="pb")
            nc.tensor.matmul(wps, lhsT=warm[:, 0:P], rhs=warm,
                             start=True, stop=True)

        def x_sl(kc, h):
            return xt[:, N * kc + NHALF * h:N * kc + NHALF * (h + 1)]

        # ---- stage 1: qkvT chunks --------------------------------------
        qkvT = [None] * 9

        def stage1_chunk(s, h):
            m = STAGE_M[s]
            if qkvT[m] is None:
                qkvT[m] = qkvpool.tile([P, N], bf16, name=f"qkvT{m}")
            ps = mmpool.tile([P, NHALF], f32, tag="mm")
            for kc in range(KC):
                nc.tensor.matmul(
                    ps,
                    lhsT=wqt[:, KC * P * s + P * kc:KC * P * s + P * (kc + 1)],
                    rhs=x_sl(kc, h),
                    start=(kc == 0), stop=(kc == KC - 1),
                )
            dst = qkvT[m][:, NHALF * h:NHALF * (h + 1)]
            if s >= 6:
                nc.vector.tensor_copy(dst, ps)
            else:
                nc.scalar.copy(dst, ps)

        for s in range(6):  # q and k chunks
            stage1_chunk(s, 0)
            stage1_chunk(s, 1)

        # ---- DVE: band products (emitted early; run as soon as q/k land)
        def make_prod(off, kc):
            q = qkvT[kc]
            k = qkvT[3 + kc]
            pr = prodpool.tile([P, N], bf16, tag="prod",
                               name=f"prod{off}_{kc}")
            if off == 0:
                # col 0 zeroed: result masked after exp anyway
                nc.vector.memset(pr[:, 0:1], 0.0)
                nc.vector.tensor_mul(pr[:, 1:], q[:, 1:], k[:, 0:N - 1])
            elif off == 1:
                nc.vector.tensor_mul(pr, q, k)
            else:
                # col N-1 zeroed: result masked after exp anyway
                nc.vector.memset(pr[:, N - 1:N], 0.0)
                nc.vector.tensor_mul(pr[:, 0:N - 1], q[:, 0:N - 1], k[:, 1:N])
            return pr

        prods = [[None] * KC for _ in range(3)]
        for kc in range(KC):
            for off in range(3):
                prods[off][kc] = make_prod(off, kc)

        # dL[t] = v[t-1] - v[t] (padded); attn = v + p_l*dL - p_r*shift(dL)
        # dL chains run on Pool (SBUF-only work; GPSIMD cannot touch PSUM)
        dLs = [None] * KC

        def make_dL(kc):
            v = qkvT[6 + kc]
            dL = avpool.tile([P, N + 1], bf16, tag="dv", bufs=3,
                             name=f"dL{kc}")
            nc.gpsimd.memset(dL[:, 0:1], 0.0)
            nc.gpsimd.memset(dL[:, N:N + 1], 0.0)
            nc.gpsimd.tensor_sub(dL[:, 1:N], v[:, 0:N - 1], v[:, 1:N])
            dLs[kc] = dL

        # e stripes: off0 at partition 0 and off1 at partition 32 of eA;
        # off2 in its own base-0 tile eB (so every DVE tensor-tensor has
        # matching base partitions; only ACT ops shift partitions, which
        # the hardware supports). The softmax denominator is then two
        # accumulated indicator matmuls (eA stripes + eB).
        e_a, e_b = [], []
        for h in range(2):
            ea = epool.tile([38, NHALF], f32, tag="ea", bufs=2,
                            name=f"ea{h}")
            nc.gpsimd.memset(ea, 0.0)
            e_a.append(ea)
            eb = epool.tile([H, NHALF], f32, tag="eb", bufs=2,
                            name=f"eb{h}")
            e_b.append(eb)

        scale = float(HD) ** -0.5

        def scores(h):
            for off in range(3):
                sps = spool.tile([H, NHALF], f32, tag="s")
                for kc in range(KC):
                    nc.tensor.matmul(
                        sps,
                        lhsT=ind6[:, 6 * kc:6 * (kc + 1)],
                        rhs=prods[off][kc][:, NHALF * h:NHALF * (h + 1)],
                        start=(kc == 0), stop=(kc == KC - 1),
                    )
                if off == 2:
                    dst = e_b[h]
                else:
                    dst = e_a[h][32 * off:32 * off + H, :]
                with tc.high_priority():
                    nc.scalar.activation(dst, sps, AF.Exp, scale=scale)
            # boundary mask, inline on ACT right after the exps:
            # no left neighbor at t=0 (h=0), no right neighbor at N-1 (h=1)
            with tc.high_priority():
                if h == 0:
                    nc.scalar.memzero(e_a[0][0:H, 0:1])
                else:
                    nc.scalar.memzero(e_b[1][0:H, NHALF - 1:NHALF])

        def v_chunk(s):
            stage1_chunk(s, 0)
            stage1_chunk(s, 1)

        # v chunks interleaved with score matmuls: keeps the PE busy while
        # the DVE finishes prods / the ACT runs exps
        v_chunk(6)
        scores(0)
        make_dL(0)
        v_chunk(7)
        scores(1)
        make_dL(1)
        v_chunk(8)
        make_dL(2)

        # softmax denominator: two accumulated fp32 matmuls per half
        den_ps = []
        for h in range(2):
            dps = spool.tile([H, NHALF], f32, tag="s", name=f"den{h}")
            nc.tensor.matmul(dps, lhsT=indA, rhs=e_a[h],
                             start=True, stop=False)
            nc.tensor.matmul(dps, lhsT=i6, rhs=e_b[h],
                             start=False, stop=True)
            den_ps.append(dps)

        # reciprocal + p = e * rec  (p in bf16 for the broadcast matmul)
        p_half = [[None, None] for _ in range(2)]  # [h][0 -> off0, 1 -> off2]
        recs_dbg = [None, None]

        def softmax(h):
            with tc.high_priority():
                rec = epool.tile([H, NHALF], f32, tag="rec", bufs=2)
                recs_dbg[h] = rec
                nc.vector.reciprocal_approx_fast(out=rec, in_=den_ps[h])
                for i, src_e in enumerate((e_a[h][0:H, :], e_b[h])):
                    pt = epool.tile([H, NHALF], bf16, tag="p", bufs=4,
                                    name=f"p{h}_{i}")
                    nc.vector.tensor_mul(pt, src_e, rec)
                    p_half[h][i] = pt

        softmax(0)

        # ---- p broadcast (PE) + AV (DVE reads the broadcast PSUM) ------
        attn = [aopool.tile([P, N], bf16, name=f"attn{kc}")
                for kc in range(KC)]
        ybuf = [ypool.tile([P, KC * NHALF], bf16, name=f"ybuf{h}")
                for h in range(2)]

        def bcast(h, i, kc):
            pbps = pbpool.tile([P, NHALF], f32, tag="pb")
            nc.tensor.matmul(
                pbps,
                lhsT=ind6T[:, P * kc:P * (kc + 1)],
                rhs=p_half[h][i],
                start=True, stop=True,
            )
            return pbps

        def av_chain(h, kc, pb0, pb2):
            lo = NHALF * h
            hi = lo + NHALF
            dL = dLs[kc]
            m1 = avpool.tile([P, NHALF], bf16, tag="m", bufs=4)
            nc.vector.tensor_mul(m1, pb0, dL[:, lo:hi])
            m2 = avpool.tile([P, NHALF], bf16, tag="m", bufs=4)
            nc.vector.tensor_mul(m2, pb2, dL[:, lo + 1:hi + 1])
            s12 = avpool.tile([P, NHALF], bf16, tag="m", bufs=4)
            nc.vector.tensor_sub(s12, m1, m2)
            nc.vector.tensor_add(attn[kc][:, lo:hi], s12,
                                 qkvT[6 + kc][:, lo:hi])

        def proj(h, kc, yps):
            lo = NHALF * h
            hi = lo + NHALF
            for m in range(KC):
                nc.tensor.matmul(
                    yps[m],
                    lhsT=wpt[:, KC * P * m + P * kc:KC * P * m + P * (kc + 1)],
                    rhs=attn[kc][:, lo:hi],
                    start=(kc == 0), stop=(kc == KC - 1),
                )

        for h in range(2):
            yps = [mmpool.tile([P, NHALF], f32, tag="mm", name=f"y{m}_{h}")
                   for m in range(KC)]
            pb = {}
            pb[0] = (bcast(h, 0, 0), bcast(h, 1, 0))
            pb[1] = (bcast(h, 0, 1), bcast(h, 1, 1))
            av_chain(h, 0, *pb[0])
            if h == 0:
                softmax(1)
            proj(h, 0, yps)
            pb[2] = (bcast(h, 0, 2), bcast(h, 1, 2))
            av_chain(h, 1, *pb[1])
            proj(h, 1, yps)
            av_chain(h, 2, *pb[2])
            proj(h, 2, yps)

            for m in range(KC):
                nc.scalar.add(ybuf[h][:, NHALF * m:NHALF * (m + 1)], yps[m],
                              biasT[:, m:m + 1])
            # one output DMA per half: dst rows 128m+p, cols [512h, 512h+512)
            nc.sync.dma_start(
                out=d_yT.rearrange("(m p) n -> p m n", p=P)[
                    :, :, NHALF * h:NHALF * (h + 1)],
                in_=ybuf[h].rearrange("p (m n) -> p m n", m=KC))

        if KDEBUG:
            for h in range(2):
                nc.sync.dma_start(out=d_dbg_e[h][0:38, :], in_=e_a[h])
                nc.sync.dma_start(out=d_dbg_e[h][38:44, :], in_=e_b[h])
                nc.sync.dma_start(out=d_dbg_rec[h], in_=recs_dbg[h])
                nc.sync.dma_start(out=d_dbg_p[h][0:H, :],
                                  in_=p_half[h][0])
                nc.sync.dma_start(out=d_dbg_p[h][H:2 * H, :],
                                  in_=p_half[h][1])
            for kc in range(KC):
                nc.sync.dma_start(out=d_dbg_attn[P * kc:P * (kc + 1), :],
                                  in_=attn[kc])
            for m in range(9):
                nc.sync.dma_start(out=d_dbg_qkv[P * m:P * (m + 1), :],
                                  in_=qkvT[m])

    nc.compile()
    return nc


def _host_inputs(x, qkv_w, proj_w, proj_b):
    import ml_dtypes
    bf = ml_dtypes.bfloat16

    qkv_w = qkv_w.astype(np.float32)
    proj_w = proj_w.astype(np.float32)

    # wq packed per stage chunk: [p, s*384 + kc*128 + i]
    #   = qkv_w[128*STAGE_M[s] + i, 128*kc + p]
    wq = np.empty((P, 9 * KC * P), np.float32)
    for s, m in enumerate(STAGE_M):
        blk = qkv_w[P * m:P * (m + 1), :]  # [i=128, c=384]
        t = blk.T.reshape(KC, P, P).transpose(1, 0, 2).reshape(P, KC * P)
        wq[:, KC * P * s:KC * P * (s + 1)] = t
    wp = np.empty((P, KC * KC * P), np.float32)
    for m in range(KC):
        blk = proj_w[P * m:P * (m + 1), :]
        t = blk.T.reshape(KC, P, P).transpose(1, 0, 2).reshape(P, KC * P)
        wp[:, KC * P * m:KC * P * (m + 1)] = t

    ind6 = np.zeros((P, 6 * KC), np.float32)
    ind6T = np.zeros((H, P * KC), np.float32)
    for kc in range(KC):
        for p in range(P):
            ind6[p, 6 * kc + 2 * kc + p // HD] = 1.0
            ind6T[2 * kc + p // HD, P * kc + p] = 1.0
    indA = np.zeros((38, H), np.float32)
    for off in range(2):
        for j in range(H):
            indA[32 * off + j, j] = 1.0
    i6 = np.eye(H, dtype=np.float32)
    biasT = proj_b.astype(np.float32).reshape(KC, P).T.copy()

    shared = {
        "wqp": wq.astype(bf),
        "wpp": wp.astype(bf),
        "ind6": ind6.astype(bf),
        "ind6T": ind6T.astype(bf),
        "indA": indA,
        "i6": i6,
        "biasT": np.ascontiguousarray(biasT),
    }
    in_maps = []
    for b in range(B):
        m = dict(shared)
        xT = x[b].astype(np.float32).T  # [C, N]
        m["xp"] = np.ascontiguousarray(
            xT.reshape(KC, P, N).transpose(1, 0, 2).reshape(P, KC * N)
        ).astype(bf)
        in_maps.append(m)
    return in_maps


def kernel(x, qkv_w, proj_w, proj_b, _trace=False):
    from concourse import bass_utils

    x = np.asarray(x)
    if "nc" not in _cached:
        _cached["nc"] = _build_nc()
    nc = _cached["nc"]
    in_maps = _host_inputs(x, np.asarray(qkv_w), np.asarray(proj_w),
                           np.asarray(proj_b))
    res = bass_utils.run_bass_kernel_spmd(
        nc, in_maps, core_ids=list(range(NCORES)), trace=_trace)
    out = np.empty((B, N, C), np.float32)
    for b in range(B):
        out[b] = res.results[b]["yT"].astype(np.float32).T
    if _trace:
        _cached["last_result"] = res
    return out
